# revision 24
# baseline (speedup 1.0000x reference)
"""Distributed multi-head attention kernel for 8 TRN2 NeuronCores.

Module: B=2, N=2048, D_MODEL=1024, H=16, D_HEAD=64 attention with
arbitrary rotary embedding, key-side boolean masking, softmax, and
output projection.

Sharding: head-parallel attention (2 heads per core, both batches),
then one AllToAll per (batch, q-half) to a striped row-parallel layout
for the output projection. Core c owns q-rows [qh*1024 + c*128 ...+128]
of each (batch, q-half) -> 4 x 128 = 512 output rows per core.

Key design points:
 - All matmuls bf16 with fp32 PSUM accumulation.
 - Every matmul lhsT has 128 columns so the compiler's Fast Weight
   Load path stays enabled (65-col weights serialize LDWEIGHTS with
   the matmul, punching holes in PE activity that re-throttle the
   HAM clock gate to K=4/8 for the whole attention phase).
 - attn@v lhsT per (key tile, head) = [v_h(64) | ones(1) | zeros(63)],
   M=128: rows 0..63 = head output, row 64 = softmax denominator.
 - qT/kT produced in [chan, row] layout so scores come out transposed
   [keys, qrows] with keys on partitions.
 - Rotary via host-rotated weight copies: rot2(x@W) == x@Wr.
 - Key mask folded into the softmax exp as a per-partition bias.
 - Attention is software-pipelined by one key tile (scores(kt),
   exp(kt), av(kt-1)) so the PE never waits on the exp of scores it
   just produced; ACT runs back-to-back.
 - One AllToAll per (batch, q-half): [8 shards x 130, 128] bf16 with
   zero wasted shards; the first three overlap compute, only the last
   (~266KB) sits on the tail.
 - Tail: denominator reciprocal broadcast via DRAM round-trip DMA
   (stride-0 partition replication), output bias folded into the
   projection as a ones-row x bias-row accumulation chunk.
"""
import os
import warnings

warnings.filterwarnings("ignore")
import numpy as np
import ml_dtypes

from concourse import bacc, tile, mybir, bass_utils

B, N, DM, H, DH = 2, 2048, 1024, 16, 64
R = B * N
NCORES = 8
HPC = 2
CPC = HPC * DH       # 128 chans per core
KT = 8               # contraction tiles over d_model
RB = 8               # row blocks of 512 over R
NKEYT = 16           # key tiles of 128 over N
QHS = 1024           # qrows per (batch, q-half) unit
NUNITS = B * (N // QHS)  # 4 (b, qh) units
SHARD_ROWS = 2 * (DH + 1)  # 130: [hA 64 | denA 1 | hB 64 | denB 1]

F32 = mybir.dt.float32
BF16 = mybir.dt.bfloat16

LAST_EXEC_TIME_NS = None
LAST_TRACE_DIR = None


def _install_trace_shim():
    import sys
    import types
    import ctypes
    import contextlib

    if "antenv.axon_hooks" in sys.modules:
        return
    so_path = "/opt/axon/libaxon_pjrt.so"
    hook = None
    if os.path.exists(so_path):
        lib = ctypes.CDLL(so_path)
        if hasattr(lib, "axon_start_nrt_profile"):
            lib.axon_start_nrt_profile.argtypes = [
                ctypes.POINTER(ctypes.c_int64), ctypes.c_size_t]
            lib.axon_start_nrt_profile.restype = ctypes.c_int64
            lib.axon_stop_nrt_profile.argtypes = [ctypes.c_char_p]
            lib.axon_stop_nrt_profile.restype = ctypes.c_int64

            @contextlib.contextmanager
            def _hook(output_dir, device_ids):
                import jax
                jax.devices()
                if device_ids:
                    ids = (ctypes.c_int64 * len(device_ids))(*device_ids)
                    rc = lib.axon_start_nrt_profile(ids, len(device_ids))
                else:
                    rc = lib.axon_start_nrt_profile(None, 0)
                if rc != 0:
                    raise RuntimeError(f"axon_start_nrt_profile rc={rc}")
                try:
                    yield
                finally:
                    n = lib.axon_stop_nrt_profile(str(output_dir).encode())
                    print(f"[trace] {n} profile file(s) -> {output_dir}")

            hook = _hook

    mod = types.ModuleType("antenv.axon_hooks")
    mod.get_axon_ntff_profile_hook = lambda: hook
    mod.set_axon_ntff_profile_hook = lambda h: None
    sys.modules["antenv.axon_hooks"] = mod
    bass_utils.upload_artifacts = lambda tmpdir: tmpdir


def _rot_cols(w):
    wr = np.empty_like(w)
    wr[:, 0::2] = -w[:, 1::2]
    wr[:, 1::2] = w[:, 0::2]
    return wr


def build(dbg=False):
    nc = bacc.Bacc("TRN2", target_bir_lowering=False, debug=False,
                   num_devices=NCORES)

    xt_d = nc.dram_tensor("xt", [DM, R], BF16, kind="ExternalInput")
    wq_d = nc.dram_tensor("wq", [DM, CPC], BF16, kind="ExternalInput")
    wqr_d = nc.dram_tensor("wqr", [DM, CPC], BF16, kind="ExternalInput")
    wk_d = nc.dram_tensor("wk", [DM, CPC], BF16, kind="ExternalInput")
    wkr_d = nc.dram_tensor("wkr", [DM, CPC], BF16, kind="ExternalInput")
    wv_d = nc.dram_tensor("wv", [DM, CPC], BF16, kind="ExternalInput")
    wout_d = nc.dram_tensor("wout", [DM, DM], BF16, kind="ExternalInput")
    boutr_d = nc.dram_tensor("boutr", [1, DM], BF16, kind="ExternalInput")
    cost_d = nc.dram_tensor("cost", [CPC, N], BF16, kind="ExternalInput")
    sint_d = nc.dram_tensor("sint", [CPC, N], BF16, kind="ExternalInput")
    maskb_d = nc.dram_tensor("maskb", [128, R // 128], F32, kind="ExternalInput")

    out_d = nc.dram_tensor("out", [NUNITS * 128, DM], F32,
                           kind="ExternalOutput")

    a2a_in = [nc.dram_tensor(f"a2a_in{u}", [NCORES * SHARD_ROWS, 128], BF16)
              for u in range(NUNITS)]
    a2a_out = [nc.dram_tensor(f"a2a_out{u}", [NCORES * SHARD_ROWS, 128], BF16)
               for u in range(NUNITS)]
    recip_d = nc.dram_tensor("recipd", [2 * NCORES, NUNITS * 128], BF16)
    dumm_in = nc.dram_tensor("dummin", [NCORES * 2, 2], BF16)
    dumm_out = nc.dram_tensor("dummout", [NCORES * 2, 2], BF16)

    with tile.TileContext(nc) as tc:
        with tc.tile_pool(name="persist", bufs=1) as pp:
            wq_sb = pp.tile([128, KT, CPC], BF16, tag="wq")
            wqr_sb = pp.tile([128, KT, CPC], BF16, tag="wqr")
            wk_sb = pp.tile([128, KT, CPC], BF16, tag="wk")
            wkr_sb = pp.tile([128, KT, CPC], BF16, tag="wkr")
            wv_sb = pp.tile([128, KT, CPC], BF16, tag="wv")
            cost_sb = pp.tile([CPC, N], BF16, tag="cost")
            sint_sb = pp.tile([CPC, N], BF16, tag="sint")
            maskb_sb = pp.tile([128, R // 128], F32, tag="maskb")
            qt_sb = pp.tile([CPC, R], BF16, tag="qt")
            kt_sb = pp.tile([CPC, R], BF16, tag="kt")
            # [128 keys, key-tile g, head, 128]: cols = [v_h | 1 | 0...]
            vaug_sb = pp.tile([128, B * NKEYT, HPC, 128], BF16, tag="vaug")
            wo_sb = pp.tile([128, KT, DM], BF16, tag="wo")
            boutr_sb = pp.tile([1, DM], BF16, tag="boutr")
            ones1_sb = pp.tile([1, 128], BF16, tag="ones1")

            def ktview(d):
                return d.ap().rearrange("(k p) n -> p k n", p=128)

            xt_view = xt_d.ap().rearrange("(k p) n -> p k n", p=128)

            # first xt block + weights first so matmuls start early;
            # per-kt pieces across both queues so matmul #0 only waits for
            # its own contraction slice
            xt_sb0 = pp.tile([128, KT, 512], BF16, tag="xt0")
            for kt in range(KT):
                eng = nc.sync if kt % 2 == 0 else nc.scalar
                eng.dma_start(xt_sb0[:, kt, :], xt_view[:, kt, 0:512])
            nc.sync.dma_start(wq_sb[:], ktview(wq_d))
            nc.scalar.dma_start(wqr_sb[:], ktview(wqr_d))
            nc.sync.dma_start(wk_sb[:], ktview(wk_d))
            nc.scalar.dma_start(wkr_sb[:], ktview(wkr_d))
            nc.sync.dma_start(wv_sb[:], ktview(wv_d))
            # zero vaug (cols 65..127 must be 0), set the ones column
            nc.vector.memset(vaug_sb[:], 0.0)
            nc.vector.memset(vaug_sb[:, :, :, DH:DH + 1], 1.0)
            nc.vector.memset(ones1_sb[:], 1.0)
            # all-zero weights for no-op PE bridge matmuls (accumulate +0)
            zw_sb = pp.tile([128, 128], BF16, tag="zw")
            nc.vector.memset(zw_sb[:], 0.0)
            # dummy collective: absorbs the ~11us first-trigger warmup
            # delay on the CC stream while phase-1 DMAs run
            dz_sb = pp.tile([NCORES * 2, 2], BF16, tag="dz")
            nc.vector.memset(dz_sb[:], 0.0)
            nc.sync.dma_start(dumm_in[:, :], dz_sb[:])
            nc.gpsimd.collective_compute(
                "AllToAll", mybir.AluOpType.bypass,
                replica_groups=[list(range(NCORES))],
                ins=[dumm_in.ap().opt()], outs=[dumm_out.ap().opt()])
            # pre-load the ACT Exp table during the initial DMA wait so the
            # first real softmax exp doesn't stall the pipeline
            warm_sb = pp.tile([1, 2], F32, tag="warm")
            nc.vector.memset(warm_sb[:], 0.0)
            nc.scalar.activation(warm_sb[0:1, 1:2], warm_sb[0:1, 0:1],
                                 mybir.ActivationFunctionType.Exp)
            nc.scalar.dma_start(cost_sb[:], cost_d[:, :])
            nc.scalar.dma_start(sint_sb[:], sint_d[:, :])
            nc.scalar.dma_start(maskb_sb[:], maskb_d[:, :])

            # ---- Phase 1: projections + rotary + v_aug ----
            with tc.tile_pool(name="p1", bufs=2) as p1, \
                 tc.tile_pool(name="ps1", bufs=1, space="PSUM") as ps1:
                for rb in range(RB):
                    c0 = rb * 512
                    if rb == 0:
                        xt_sb = xt_sb0
                    else:
                        xt_sb = p1.tile([128, KT, 512], BF16, tag="xt")
                        if rb == 4:
                            eng = nc.gpsimd
                        elif rb % 2 == 1:
                            eng = nc.sync
                        else:
                            eng = nc.scalar
                        eng.dma_start(xt_sb[:], xt_view[:, :, c0:c0 + 512])

                    q_ps = ps1.tile([128, 512], F32, tag="q")
                    qr_ps = ps1.tile([128, 512], F32, tag="qr")
                    k_ps = ps1.tile([128, 512], F32, tag="k")
                    kr_ps = ps1.tile([128, 512], F32, tag="kr")
                    v_ps = ps1.tile([128, 512], F32, tag="v")
                    for kt in range(KT):
                        st, sp = kt == 0, kt == KT - 1
                        for ps_t, w_t in [(q_ps, wq_sb), (qr_ps, wqr_sb),
                                          (k_ps, wk_sb), (kr_ps, wkr_sb)]:
                            nc.tensor.matmul(ps_t[:], w_t[:, kt, :],
                                             xt_sb[:, kt, :], start=st, stop=sp)
                        for vt in range(4):
                            nc.tensor.matmul(
                                v_ps[:, vt * 128:(vt + 1) * 128],
                                xt_sb[:, kt, vt * 128:(vt + 1) * 128],
                                wv_sb[:, kt, :], start=(st and vt == 0), stop=sp)

                    cc = c0 % N
                    tmp = p1.tile([128, 512], BF16, tag="rottmp")
                    for dst, a_ps, b_ps in [(qt_sb, q_ps, qr_ps),
                                            (kt_sb, k_ps, kr_ps)]:
                        dv = dst[:, c0:c0 + 512]
                        nc.vector.tensor_mul(dv, a_ps[:], cost_sb[:, cc:cc + 512])
                        nc.vector.tensor_mul(tmp[:], b_ps[:], sint_sb[:, cc:cc + 512])
                        nc.vector.tensor_add(dv, dv, tmp[:])

                    b = rb // 4
                    kt0 = rb * 4
                    vp = v_ps[:].rearrange("p (t c) -> p t c", c=128)
                    nc.vector.tensor_copy(
                        vaug_sb[:, kt0:kt0 + 4, 0, 0:DH], vp[:, :, 0:DH])
                    nc.vector.tensor_copy(
                        vaug_sb[:, kt0:kt0 + 4, 1, 0:DH], vp[:, :, DH:2 * DH])

                # keep PE busy across the phase transition (an idle gap
                # here re-throttles the PE clock for the rest of the run)
                brid_ps = ps1.tile([128, 512], F32, tag="brid")
                for i in range(12):
                    nc.tensor.matmul(brid_ps[:], wq_sb[:, i % KT, :],
                                     xt_sb0[:, i % KT, :],
                                     start=(i == 0), stop=(i == 11))

            # wout needed only in phase 3 — load it behind phase-1 traffic
            nc.scalar.dma_start(wo_sb[:], wout_d.ap().rearrange(
                "(k p) n -> p k n", p=128))
            nc.sync.dma_start(boutr_sb[:], boutr_d[:, :])

            # ---- per-unit tail prep: gather + normalize, SBUF/DMA only.
            # prep(u) is emitted one unit late so its DMAs never
            # head-of-line-block attention; nothing here touches the ACT
            # queue (phase-2 exps must stream uninterrupted) or PSUM.
            onorm_tiles = [None] * NUNITS

            def tail_prep(u):
                av = a2a_out[u].ap().rearrange("(j r) n -> r j n",
                                               r=SHARD_ROWS)
                o_t = pp.tile([128, NCORES, 128], BF16, tag=f"ot{u}",
                              name=f"ot{u}")
                den_t = pp.tile([2 * NCORES, 128], BF16, tag=f"dt{u}",
                                name=f"dt{u}")
                nc.sync.dma_start(den_t[0:NCORES, :], av[DH:DH + 1, :, :])
                nc.sync.dma_start(den_t[NCORES:2 * NCORES, :],
                                  av[CPC + 1:CPC + 2, :, :])
                nc.sync.dma_start(o_t[0:DH, :, :], av[0:DH, :, :])
                oeng = nc.gpsimd if u == NUNITS - 1 else nc.sync
                oeng.dma_start(o_t[DH:CPC, :, :], av[DH + 1:CPC + 1, :, :])
                recipf = pp.tile([2 * NCORES, 128], F32, tag=f"rf{u}",
                                 name=f"rf{u}")
                recipb16 = pp.tile([2 * NCORES, 128], BF16, tag=f"rb{u}",
                                   name=f"rb{u}")
                nc.vector.reciprocal(recipf[:], den_t[:])
                nc.vector.tensor_copy(recipb16[:], recipf[:])
                nc.sync.dma_start(recip_d[:, u * 128:(u + 1) * 128],
                                  recipb16[:])
                recipb = pp.tile([128, NCORES, 128], BF16, tag=f"rc{u}",
                                 name=f"rc{u}")
                rv = recip_d.ap()
                for j in range(NCORES):
                    for h in range(HPC):
                        eng = nc.gpsimd if (u == NUNITS - 1 and
                                            (2 * j + h) % 2 == 1) else nc.sync
                        eng.dma_start(
                            recipb[h * DH:(h + 1) * DH, j, :],
                            rv[NCORES * h + j:NCORES * h + j + 1,
                               u * 128:(u + 1) * 128]
                            .to_broadcast((DH, 128)))
                onorm = pp.tile([128, NCORES, 128], BF16, tag=f"on{u}",
                                name=f"on{u}")
                nc.vector.tensor_mul(onorm[:], o_t[:], recipb[:])
                onorm_tiles[u] = onorm

            # ---- Phase 2: attention, two heads packed, per (b, q-half) ----
            with tc.tile_pool(name="p2", bufs=2) as p2, \
                 tc.tile_pool(name="ps_sc", bufs=1, space="PSUM") as ps_sc, \
                 tc.tile_pool(name="ps_o", bufs=1, space="PSUM") as ps_o:
                for u in range(NUNITS):
                    b, qh = u // 2, u % 2
                    qbase = b * N + qh * QHS
                    o_ps = [ps_o.tile([128, QHS], F32, tag=f"outp{h}",
                                      name=f"ops{h}") for h in range(HPC)]
                    # software-pipelined by one key tile: emit scores(kt),
                    # exp(kt), av(kt-1) so the PE never waits on the exp of
                    # scores it just produced; ACT stays back-to-back.
                    prev_p = None
                    for kt in range(NKEYT + 1):
                        if kt < NKEYT:
                            g = b * NKEYT + kt
                            krow = b * N + kt * 128
                            sc = [ps_sc.tile([128, QHS], F32, tag=f"sc{h}",
                                             name=f"sc{h}") for h in range(HPC)]
                            # interleave heads so the PE runs them in
                            # different row groups concurrently
                            for qq in range(QHS // 512):
                                for h in range(HPC):
                                    ho = h * DH
                                    nc.tensor.matmul(
                                        sc[h][:, qq * 512:(qq + 1) * 512],
                                        kt_sb[ho:ho + DH, krow:krow + 128],
                                        qt_sb[ho:ho + DH,
                                              qbase + qq * 512:qbase + (qq + 1) * 512],
                                        start=True, stop=True)
                            p_sb = []
                            for h in range(HPC):
                                pt = p2.tile([128, QHS], BF16, tag=f"p{h}",
                                             name=f"pt{h}")
                                nc.scalar.activation(
                                    pt[:], sc[h][:],
                                    mybir.ActivationFunctionType.Exp,
                                    bias=maskb_sb[:, g:g + 1],
                                    scale=float(DH ** -0.5))
                                p_sb.append(pt)
                        if kt == NKEYT:
                            # no-op bridge (+0 accumulate) spans the final
                            # exp wait so the HAM clock gate stays warm
                            for i in range(4):
                                nc.tensor.matmul(
                                    o_ps[0][:, 0:512], zw_sb[:],
                                    qt_sb[:, 0:512],
                                    start=False, stop=False)
                        if prev_p is not None:
                            pkt = kt - 1
                            pg = b * NKEYT + pkt
                            for h in range(HPC):
                                va_l = vaug_sb[:, pg, h, :]
                                for qq in range(QHS // 512):
                                    nc.tensor.matmul(
                                        o_ps[h][:, qq * 512:(qq + 1) * 512],
                                        va_l,
                                        prev_p[h][:, qq * 512:(qq + 1) * 512],
                                        start=(pkt == 0),
                                        stop=(pkt == NKEYT - 1))
                        if kt < NKEYT:
                            prev_p = p_sb

                    # stage [hA|denA|hB|denB] shards and fire this unit's A2A
                    onb = [p2.tile([DH + 1, QHS], BF16, tag=f"onb{h}",
                                   name=f"onb{h}") for h in range(HPC)]
                    for h in range(HPC):
                        nc.vector.tensor_copy(onb[h][:], o_ps[h][0:DH + 1, :])
                    # staging on the SP queue only: a scalar-queue DMA here
                    # would head-of-line-block the next unit's exp stream
                    for d in range(NCORES):
                        r0 = d * SHARD_ROWS
                        cs = slice(d * 128, (d + 1) * 128)
                        nc.sync.dma_start(a2a_in[u][r0:r0 + DH + 1, :],
                                          onb[0][:, cs])
                        nc.sync.dma_start(
                            a2a_in[u][r0 + DH + 1:r0 + SHARD_ROWS, :],
                            onb[1][:, cs])
                    nc.gpsimd.collective_compute(
                        "AllToAll", mybir.AluOpType.bypass,
                        replica_groups=[list(range(NCORES))],
                        ins=[a2a_in[u].ap().opt()],
                        outs=[a2a_out[u].ap().opt()])
                    if u >= 1:
                        tail_prep(u - 1)
                tail_prep(NUNITS - 1)

            # ---- Phase 3: output projections; units 0-2 are ready the
            # moment the PSUM pools free, so the PE rolls straight from
            # attention into a dense warm matmul chain; only unit 3's
            # A2A+prep chain is exposed.
            with tc.tile_pool(name="p3", bufs=2) as p3, \
                 tc.tile_pool(name="ps3", bufs=2, space="PSUM") as ps3:
                for u in range(NUNITS):
                    y_ps = ps3.tile([128, DM], F32, tag="y")
                    for nb in range(2):
                        nsl = slice(nb * 512, (nb + 1) * 512)
                        for j in range(NCORES):
                            nc.tensor.matmul(
                                y_ps[:, nsl], onorm_tiles[u][:, j, :],
                                wo_sb[:, j, nsl],
                                start=(j == 0), stop=False)
                        nc.tensor.matmul(y_ps[:, nsl], ones1_sb[:],
                                         boutr_sb[:, nsl],
                                         start=False, stop=True)
                    y_sb = p3.tile([128, DM], F32, tag="y_sb", name=f"ysb{u}")
                    if u % 2 == 0:
                        nc.vector.tensor_copy(y_sb[:], y_ps[:])
                    else:
                        nc.scalar.copy(y_sb[:], y_ps[:])
                    eng = nc.sync if u % 2 == 0 else nc.scalar
                    eng.dma_start(out_d[u * 128:(u + 1) * 128, :], y_sb[:])

    nc.compile()
    return nc


_NC_CACHE = None


def kernel(x, mask, pos_emb, Wq, Wkv, Wout, bout):
    global LAST_EXEC_TIME_NS, LAST_TRACE_DIR, _NC_CACHE

    x = np.asarray(x, dtype=np.float32)
    mask = np.asarray(mask)
    pos_emb = np.asarray(pos_emb, dtype=np.float32)
    Wq = np.asarray(Wq, dtype=np.float32)
    Wkv = np.asarray(Wkv, dtype=np.float32)
    Wout = np.asarray(Wout, dtype=np.float32)
    bout = np.asarray(bout, dtype=np.float32)

    bf = ml_dtypes.bfloat16
    xt = np.ascontiguousarray(x.reshape(R, DM).T).astype(bf)
    wk_full = Wkv[:, :H * DH]
    wv_full = Wkv[:, H * DH:]
    cost = np.ascontiguousarray(np.tile(np.cos(pos_emb).T, (HPC, 1))).astype(bf)
    sint = np.ascontiguousarray(np.tile(np.sin(pos_emb).T, (HPC, 1))).astype(bf)
    maskb = np.ascontiguousarray(
        np.where(mask.reshape(R), 0.0, -1e5).astype(np.float32)
        .reshape(R // 128, 128).T)
    wqr = _rot_cols(Wq)
    wkr = _rot_cols(wk_full)

    in_maps = []
    for c in range(NCORES):
        cols = slice(c * CPC, (c + 1) * CPC)
        in_maps.append({
            "xt": xt,
            "wq": np.ascontiguousarray(Wq[:, cols]).astype(bf),
            "wqr": np.ascontiguousarray(wqr[:, cols]).astype(bf),
            "wk": np.ascontiguousarray(wk_full[:, cols]).astype(bf),
            "wkr": np.ascontiguousarray(wkr[:, cols]).astype(bf),
            "wv": np.ascontiguousarray(wv_full[:, cols]).astype(bf),
            "wout": Wout.astype(bf),
            "boutr": bout[None, :].astype(bf),
            "cost": cost,
            "sint": sint,
            "maskb": maskb,
        })

    dbg = bool(int(os.environ.get("BASS_KERNEL_DEBUG", "0")))
    if _NC_CACHE is None:
        _NC_CACHE = build(dbg=dbg)
    nc = _NC_CACHE

    trace = bool(int(os.environ.get("BASS_KERNEL_TRACE", "0")))
    kwargs = {}
    if trace:
        _install_trace_shim()
        tdir = os.environ.get("BASS_TRACE_DIR", "/tmp/bass_trace_out")
        os.makedirs(tdir, exist_ok=True)
        kwargs["tmpdir"] = tdir
    res = bass_utils.run_bass_kernel_spmd(
        nc, in_maps, core_ids=list(range(NCORES)), trace=trace, **kwargs)
    LAST_EXEC_TIME_NS = res.exec_time_ns
    if res.instructions_and_trace is not None:
        LAST_TRACE_DIR = res.instructions_and_trace[1]
        globals()["LAST_INSTS"] = res.instructions_and_trace[0]

    globals()["LAST_RESULTS"] = res.results
    # core c's out rows: [u*128 + r] = batch u//2, q-row (u%2)*1024 + c*128 + r
    y = np.empty((B, N, DM), dtype=np.float32)
    for c in range(NCORES):
        oc = res.results[c]["out"]
        for u in range(NUNITS):
            b, qh = u // 2, u % 2
            q0 = qh * QHS + c * 128
            y[b, q0:q0 + 128, :] = oc[u * 128:(u + 1) * 128, :]
    return y


# revision 33
# speedup vs baseline: 1.1100x; 1.1100x over previous
"""Distributed multi-head attention kernel for 8 TRN2 NeuronCores.

Module: B=2, N=2048, D_MODEL=1024, H=16, D_HEAD=64 attention with
arbitrary rotary embedding, key-side boolean masking, softmax, and
output projection.

Sharding: head-parallel attention (2 heads per core, both batches),
then one AllToAll per (batch, q-half) to a striped row-parallel layout
for the output projection. Core c owns q-rows [qh*1024 + c*128 ...+128]
of each (batch, q-half) -> 4 x 128 = 512 output rows per core.

Key design points:
 - All matmuls bf16 with fp32 PSUM accumulation.
 - Every matmul lhsT has 128 columns so the compiler's Fast Weight
   Load path stays enabled (65-col weights serialize LDWEIGHTS with
   the matmul, punching holes in PE activity that re-throttle the
   HAM clock gate to K=4/8 for the whole attention phase).
 - attn@v lhsT per (key tile, head) = [v_h(64) | ones(1) | zeros(63)],
   M=128: rows 0..63 = head output, row 64 = softmax denominator.
 - qT/kT produced in [chan, row] layout so scores come out transposed
   [keys, qrows] with keys on partitions.
 - Rotary via host-rotated weight copies: rot2(x@W) == x@Wr.
 - Key mask folded into the softmax exp as a per-partition bias.
 - Attention is software-pipelined by one key tile (scores(kt),
   exp(kt), av(kt-1)) so the PE never waits on the exp of scores it
   just produced; ACT runs back-to-back.
 - One AllToAll per (batch, q-half): [8 shards x 130, 128] bf16 with
   zero wasted shards; the first three overlap compute, only the last
   (~266KB) sits on the tail.
 - Tail: denominator reciprocal broadcast via DRAM round-trip DMA
   (stride-0 partition replication), output bias folded into the
   projection as a ones-row x bias-row accumulation chunk.
"""
import os
import warnings

warnings.filterwarnings("ignore")
import numpy as np
import ml_dtypes

from concourse import bacc, tile, mybir, bass_utils

B, N, DM, H, DH = 2, 2048, 1024, 16, 64
R = B * N
NCORES = 8
HPC = 2
CPC = HPC * DH       # 128 chans per core
KT = 8               # contraction tiles over d_model
RB = 8               # row blocks of 512 over R
NKEYT = 16           # key tiles of 128 over N
QHS = 1024           # qrows per (batch, q-half) unit
NUNITS = B * (N // QHS)  # 4 (b, qh) units
SHARD_ROWS = 2 * (DH + 1)  # 130: [hA 64 | denA 1 | hB 64 | denB 1]

F32 = mybir.dt.float32
BF16 = mybir.dt.bfloat16

LAST_EXEC_TIME_NS = None
LAST_TRACE_DIR = None


def _install_trace_shim():
    import sys
    import types
    import ctypes
    import contextlib

    if "antenv.axon_hooks" in sys.modules:
        return
    so_path = "/opt/axon/libaxon_pjrt.so"
    hook = None
    if os.path.exists(so_path):
        lib = ctypes.CDLL(so_path)
        if hasattr(lib, "axon_start_nrt_profile"):
            lib.axon_start_nrt_profile.argtypes = [
                ctypes.POINTER(ctypes.c_int64), ctypes.c_size_t]
            lib.axon_start_nrt_profile.restype = ctypes.c_int64
            lib.axon_stop_nrt_profile.argtypes = [ctypes.c_char_p]
            lib.axon_stop_nrt_profile.restype = ctypes.c_int64

            @contextlib.contextmanager
            def _hook(output_dir, device_ids):
                import jax
                jax.devices()
                if device_ids:
                    ids = (ctypes.c_int64 * len(device_ids))(*device_ids)
                    rc = lib.axon_start_nrt_profile(ids, len(device_ids))
                else:
                    rc = lib.axon_start_nrt_profile(None, 0)
                if rc != 0:
                    raise RuntimeError(f"axon_start_nrt_profile rc={rc}")
                try:
                    yield
                finally:
                    n = lib.axon_stop_nrt_profile(str(output_dir).encode())
                    print(f"[trace] {n} profile file(s) -> {output_dir}")

            hook = _hook

    mod = types.ModuleType("antenv.axon_hooks")
    mod.get_axon_ntff_profile_hook = lambda: hook
    mod.set_axon_ntff_profile_hook = lambda h: None
    sys.modules["antenv.axon_hooks"] = mod
    bass_utils.upload_artifacts = lambda tmpdir: tmpdir


def _rot_cols(w):
    wr = np.empty_like(w)
    wr[:, 0::2] = -w[:, 1::2]
    wr[:, 1::2] = w[:, 0::2]
    return wr


def build(dbg=False):
    nc = bacc.Bacc("TRN2", target_bir_lowering=False, debug=False,
                   num_devices=NCORES)

    xt_d = nc.dram_tensor("xt", [DM, R], BF16, kind="ExternalInput")
    wq_d = nc.dram_tensor("wq", [DM, CPC], BF16, kind="ExternalInput")
    wqr_d = nc.dram_tensor("wqr", [DM, CPC], BF16, kind="ExternalInput")
    wk_d = nc.dram_tensor("wk", [DM, CPC], BF16, kind="ExternalInput")
    wkr_d = nc.dram_tensor("wkr", [DM, CPC], BF16, kind="ExternalInput")
    wv_d = nc.dram_tensor("wv", [DM, CPC], BF16, kind="ExternalInput")
    wout_d = nc.dram_tensor("wout", [DM, DM], BF16, kind="ExternalInput")
    boutr_d = nc.dram_tensor("boutr", [1, DM], BF16, kind="ExternalInput")
    cost_d = nc.dram_tensor("cost", [CPC, N], BF16, kind="ExternalInput")
    sint_d = nc.dram_tensor("sint", [CPC, N], BF16, kind="ExternalInput")
    maskb_d = nc.dram_tensor("maskb", [128, R // 128], F32, kind="ExternalInput")

    out_d = nc.dram_tensor("out", [NUNITS * 128, DM], F32,
                           kind="ExternalOutput")

    a2a_in = [nc.dram_tensor(f"a2a_in{u}", [NCORES * SHARD_ROWS, 128], BF16)
              for u in range(NUNITS)]
    a2a_out = [nc.dram_tensor(f"a2a_out{u}", [NCORES * SHARD_ROWS, 128], BF16)
               for u in range(NUNITS)]
    recip_d = nc.dram_tensor("recipd", [2 * NCORES, NUNITS * 128], BF16)
    dumm_in = nc.dram_tensor("dummin", [NCORES * 2, 2], BF16)
    dumm_out = nc.dram_tensor("dummout", [NCORES * 2, 2], BF16)

    with tile.TileContext(nc) as tc:
        with tc.tile_pool(name="persist", bufs=1) as pp:
            wq_sb = pp.tile([128, KT, CPC], BF16, tag="wq")
            wqr_sb = pp.tile([128, KT, CPC], BF16, tag="wqr")
            wk_sb = pp.tile([128, KT, CPC], BF16, tag="wk")
            wkr_sb = pp.tile([128, KT, CPC], BF16, tag="wkr")
            wv_sb = pp.tile([128, KT, CPC], BF16, tag="wv")
            cost_sb = pp.tile([CPC, N], BF16, tag="cost")
            sint_sb = pp.tile([CPC, N], BF16, tag="sint")
            maskb_sb = pp.tile([128, R // 128], F32, tag="maskb")
            # per-batch tiles: unit-0 scores must not carry a whole-tile
            # dependency on batch-1's projections
            qt_sb = [pp.tile([CPC, N], BF16, tag=f"qt{b}", name=f"qt{b}")
                     for b in range(B)]
            kt_sb = [pp.tile([CPC, N], BF16, tag=f"kt{b}", name=f"kt{b}")
                     for b in range(B)]
            # [128 keys, key-tile, head, 128]: cols = [v_h | 1 | 0...]
            vaug_sb = [pp.tile([128, NKEYT, HPC, 128], BF16, tag=f"vaug{b}",
                               name=f"vaug{b}") for b in range(B)]
            wo_sb = pp.tile([128, KT, DM], BF16, tag="wo")
            boutr_sb = pp.tile([1, DM], BF16, tag="boutr")
            ones1_sb = pp.tile([1, 128], BF16, tag="ones1")

            def ktview(d):
                return d.ap().rearrange("(k p) n -> p k n", p=128)

            xt_view = xt_d.ap().rearrange("(k p) n -> p k n", p=128)

            # first xt block + weights first so matmuls start early;
            # per-kt pieces across both queues so matmul #0 only waits for
            # its own contraction slice
            xt_sb0 = pp.tile([128, KT, 512], BF16, tag="xt0")
            for kt in range(KT):
                eng = nc.sync if kt % 2 == 0 else nc.scalar
                eng.dma_start(xt_sb0[:, kt, :], xt_view[:, kt, 0:512])
            nc.sync.dma_start(wq_sb[:], ktview(wq_d))
            nc.scalar.dma_start(wqr_sb[:], ktview(wqr_d))
            nc.sync.dma_start(wk_sb[:], ktview(wk_d))
            nc.scalar.dma_start(wkr_sb[:], ktview(wkr_d))
            nc.sync.dma_start(wv_sb[:], ktview(wv_d))
            # zero vaug (cols 65..127 must be 0), set the ones column
            for b in range(B):
                nc.vector.memset(vaug_sb[b][:], 0.0)
                nc.vector.memset(vaug_sb[b][:, :, :, DH:DH + 1], 1.0)
            nc.vector.memset(ones1_sb[:], 1.0)
            # all-zero weights for no-op PE bridge matmuls (accumulate +0)
            zw_sb = pp.tile([128, 128], BF16, tag="zw")
            nc.vector.memset(zw_sb[:], 0.0)
            # dummy collective: absorbs the ~11us first-trigger warmup
            # delay on the CC stream while phase-1 DMAs run
            dz_sb = pp.tile([NCORES * 2, 2], BF16, tag="dz")
            nc.vector.memset(dz_sb[:], 0.0)
            nc.sync.dma_start(dumm_in[:, :], dz_sb[:])
            nc.gpsimd.collective_compute(
                "AllToAll", mybir.AluOpType.bypass,
                replica_groups=[list(range(NCORES))],
                ins=[dumm_in.ap().opt()], outs=[dumm_out.ap().opt()])
            # pre-load the ACT Exp table during the initial DMA wait so the
            # first real softmax exp doesn't stall the pipeline
            warm_sb = pp.tile([1, 2], F32, tag="warm")
            nc.vector.memset(warm_sb[:], 0.0)
            nc.scalar.activation(warm_sb[0:1, 1:2], warm_sb[0:1, 0:1],
                                 mybir.ActivationFunctionType.Exp)
            nc.scalar.dma_start(cost_sb[:], cost_d[:, :])
            nc.scalar.dma_start(sint_sb[:], sint_d[:, :])
            nc.scalar.dma_start(maskb_sb[:], maskb_d[:, :])

            # ---- Phase 1: projections + rotary + v_aug ----
            with tc.tile_pool(name="p1", bufs=2) as p1, \
                 tc.tile_pool(name="ps1", bufs=1, space="PSUM") as ps1:
                for rb in range(RB):
                    c0 = rb * 512
                    if rb == 0:
                        xt_sb = xt_sb0
                    else:
                        xt_sb = p1.tile([128, KT, 512], BF16, tag="xt")
                        if rb == 4:
                            eng = nc.gpsimd
                        elif rb % 2 == 1:
                            eng = nc.sync
                        else:
                            eng = nc.scalar
                        eng.dma_start(xt_sb[:], xt_view[:, :, c0:c0 + 512])

                    q_ps = ps1.tile([128, 512], F32, tag="q")
                    qr_ps = ps1.tile([128, 512], F32, tag="qr")
                    k_ps = ps1.tile([128, 512], F32, tag="k")
                    kr_ps = ps1.tile([128, 512], F32, tag="kr")
                    v_ps = ps1.tile([128, 512], F32, tag="v")
                    for kt in range(KT):
                        st, sp = kt == 0, kt == KT - 1
                        for ps_t, w_t in [(q_ps, wq_sb), (qr_ps, wqr_sb),
                                          (k_ps, wk_sb), (kr_ps, wkr_sb)]:
                            nc.tensor.matmul(ps_t[:], w_t[:, kt, :],
                                             xt_sb[:, kt, :], start=st, stop=sp)
                        for vt in range(4):
                            nc.tensor.matmul(
                                v_ps[:, vt * 128:(vt + 1) * 128],
                                xt_sb[:, kt, vt * 128:(vt + 1) * 128],
                                wv_sb[:, kt, :], start=(st and vt == 0), stop=sp)

                    b = rb // 4
                    cc = c0 % N
                    tmp = p1.tile([128, 512], BF16, tag="rottmp")
                    for dst, a_ps, b_ps in [(qt_sb[b], q_ps, qr_ps),
                                            (kt_sb[b], k_ps, kr_ps)]:
                        dv = dst[:, cc:cc + 512]
                        nc.vector.tensor_mul(dv, a_ps[:], cost_sb[:, cc:cc + 512])
                        nc.vector.tensor_mul(tmp[:], b_ps[:], sint_sb[:, cc:cc + 512])
                        nc.vector.tensor_add(dv, dv, tmp[:])

                    kt0 = (rb % 4) * 4
                    vp = v_ps[:].rearrange("p (t c) -> p t c", c=128)
                    nc.vector.tensor_copy(
                        vaug_sb[b][:, kt0:kt0 + 4, 0, 0:DH], vp[:, :, 0:DH])
                    nc.vector.tensor_copy(
                        vaug_sb[b][:, kt0:kt0 + 4, 1, 0:DH],
                        vp[:, :, DH:2 * DH])

                # keep PE busy across the phase transition (an idle gap
                # here re-throttles the PE clock for the rest of the run)
                brid_ps = ps1.tile([128, 512], F32, tag="brid")
                for i in range(12):
                    nc.tensor.matmul(brid_ps[:], wq_sb[:, i % KT, :],
                                     xt_sb0[:, i % KT, :],
                                     start=(i == 0), stop=(i == 11))

            # wout needed only in phase 3 — load it behind phase-1 traffic
            nc.scalar.dma_start(wo_sb[:], wout_d.ap().rearrange(
                "(k p) n -> p k n", p=128))
            nc.sync.dma_start(boutr_sb[:], boutr_d[:, :])

            # ---- per-unit tail prep: gather + normalize, SBUF/DMA only.
            # prep(u) is emitted one unit late so its DMAs never
            # head-of-line-block attention; nothing here touches the ACT
            # queue (phase-2 exps must stream uninterrupted) or PSUM.
            onorm_tiles = [None] * NUNITS

            def tail_prep(u):
                # units 2/3 are emitted after the last exp, so the ACT
                # queue is free to carry their DMAs too; gpsimd only for
                # unit 3 (its queue is blocked on collective completions)
                if u == NUNITS - 1:
                    engs = [nc.sync, nc.scalar, nc.gpsimd]
                elif u == NUNITS - 2:
                    engs = [nc.sync, nc.scalar]
                else:
                    engs = [nc.sync]
                ne = len(engs)
                av = a2a_out[u].ap().rearrange("(j r) n -> r j n",
                                               r=SHARD_ROWS)
                o_t = pp.tile([128, NCORES, 128], BF16, tag=f"ot{u}",
                              name=f"ot{u}")
                den_t = pp.tile([2 * NCORES, 128], BF16, tag=f"dt{u}",
                                name=f"dt{u}")
                engs[0].dma_start(den_t[0:NCORES, :], av[DH:DH + 1, :, :])
                engs[1 % ne].dma_start(den_t[NCORES:2 * NCORES, :],
                                       av[CPC + 1:CPC + 2, :, :])
                engs[0].dma_start(o_t[0:DH, :, :], av[0:DH, :, :])
                engs[2 % ne].dma_start(o_t[DH:CPC, :, :],
                                       av[DH + 1:CPC + 1, :, :])
                recipf = pp.tile([2 * NCORES, 128], F32, tag=f"rf{u}",
                                 name=f"rf{u}")
                recipb16 = pp.tile([2 * NCORES, 128], BF16, tag=f"rb{u}",
                                   name=f"rb{u}")
                nc.vector.reciprocal(recipf[:], den_t[:])
                nc.vector.tensor_copy(recipb16[:], recipf[:])
                engs[1 % ne].dma_start(recip_d[:, u * 128:(u + 1) * 128],
                                       recipb16[:])
                recipb = pp.tile([128, NCORES, 128], BF16, tag=f"rc{u}",
                                 name=f"rc{u}")
                rv = recip_d.ap()
                for j in range(NCORES):
                    for h in range(HPC):
                        engs[(2 * j + h) % ne].dma_start(
                            recipb[h * DH:(h + 1) * DH, j, :],
                            rv[NCORES * h + j:NCORES * h + j + 1,
                               u * 128:(u + 1) * 128]
                            .to_broadcast((DH, 128)))
                onorm = pp.tile([128, NCORES, 128], BF16, tag=f"on{u}",
                                name=f"on{u}")
                nc.vector.tensor_mul(onorm[:], o_t[:], recipb[:])
                onorm_tiles[u] = onorm

            # ---- Phase 2: attention, two heads packed, per (b, q-half) ----
            with tc.tile_pool(name="p2", bufs=2) as p2, \
                 tc.tile_pool(name="ps_sc", bufs=1, space="PSUM") as ps_sc, \
                 tc.tile_pool(name="ps_o", bufs=1, space="PSUM") as ps_o:
                for u in range(NUNITS):
                    b, qh = u // 2, u % 2
                    qbase = qh * QHS
                    o_ps = [ps_o.tile([128, QHS], F32, tag=f"outp{h}",
                                      name=f"ops{h}") for h in range(HPC)]
                    # software-pipelined by one key tile: emit scores(kt),
                    # exp(kt), av(kt-1) so the PE never waits on the exp of
                    # scores it just produced; ACT stays back-to-back.
                    prev_p = None
                    for kt in range(NKEYT + 1):
                        if kt < NKEYT:
                            g = b * NKEYT + kt
                            krow = kt * 128
                            sc = [ps_sc.tile([128, QHS], F32, tag=f"sc{h}",
                                             name=f"sc{h}") for h in range(HPC)]
                            # interleave heads so the PE runs them in
                            # different row groups concurrently
                            for qq in range(QHS // 512):
                                for h in range(HPC):
                                    ho = h * DH
                                    nc.tensor.matmul(
                                        sc[h][:, qq * 512:(qq + 1) * 512],
                                        kt_sb[b][ho:ho + DH, krow:krow + 128],
                                        qt_sb[b][ho:ho + DH,
                                                 qbase + qq * 512:qbase + (qq + 1) * 512],
                                        start=True, stop=True)
                            p_sb = []
                            for h in range(HPC):
                                pt = p2.tile([128, QHS], BF16, tag=f"p{h}",
                                             name=f"pt{h}")
                                nc.scalar.activation(
                                    pt[:], sc[h][:],
                                    mybir.ActivationFunctionType.Exp,
                                    bias=maskb_sb[:, g:g + 1],
                                    scale=float(DH ** -0.5))
                                p_sb.append(pt)
                        if kt == NKEYT or (u == 0 and 1 <= kt <= 3):
                            # no-op bridge (+0 accumulate) spans the final
                            # exp wait (and unit-0's ramp-in) so the HAM
                            # clock gate stays warm
                            for i in range(4 if kt == NKEYT else 2):
                                nc.tensor.matmul(
                                    o_ps[0][:, 0:512], zw_sb[:],
                                    qt_sb[0][:, 0:512],
                                    start=False, stop=False)
                        if prev_p is not None:
                            pkt = kt - 1
                            for h in range(HPC):
                                va_l = vaug_sb[b][:, pkt, h, :]
                                for qq in range(QHS // 512):
                                    nc.tensor.matmul(
                                        o_ps[h][:, qq * 512:(qq + 1) * 512],
                                        va_l,
                                        prev_p[h][:, qq * 512:(qq + 1) * 512],
                                        start=(pkt == 0),
                                        stop=(pkt == NKEYT - 1))
                        if kt < NKEYT:
                            prev_p = p_sb

                    # stage [hA|denA|hB|denB] shards and fire this unit's A2A
                    onb = [p2.tile([DH + 1, QHS], BF16, tag=f"onb{h}",
                                   name=f"onb{h}") for h in range(HPC)]
                    for h in range(HPC):
                        nc.vector.tensor_copy(onb[h][:], o_ps[h][0:DH + 1, :])
                    # staging off the scalar queue while exps still stream
                    # (a scalar-queue DMA would head-of-line-block them);
                    # the last unit uses the now-idle ACT queue for speed
                    seng = nc.scalar if u == NUNITS - 1 else nc.sync
                    for d in range(NCORES):
                        r0 = d * SHARD_ROWS
                        cs = slice(d * 128, (d + 1) * 128)
                        seng.dma_start(a2a_in[u][r0:r0 + DH + 1, :],
                                       onb[0][:, cs])
                        seng.dma_start(
                            a2a_in[u][r0 + DH + 1:r0 + SHARD_ROWS, :],
                            onb[1][:, cs])
                    nc.gpsimd.collective_compute(
                        "AllToAll", mybir.AluOpType.bypass,
                        replica_groups=[list(range(NCORES))],
                        ins=[a2a_in[u].ap().opt()],
                        outs=[a2a_out[u].ap().opt()])
                    if u >= 1:
                        tail_prep(u - 1)
                tail_prep(NUNITS - 1)

            # ---- Phase 3: output projections; units 0-2 are ready the
            # moment the PSUM pools free, so the PE rolls straight from
            # attention into a dense warm matmul chain; only unit 3's
            # A2A+prep chain is exposed.
            with tc.tile_pool(name="p3", bufs=2) as p3, \
                 tc.tile_pool(name="ps3", bufs=2, space="PSUM") as ps3:
                for u in range(NUNITS):
                    y_ps = ps3.tile([128, DM], F32, tag="y")
                    for nb in range(2):
                        nsl = slice(nb * 512, (nb + 1) * 512)
                        for j in range(NCORES):
                            nc.tensor.matmul(
                                y_ps[:, nsl], onorm_tiles[u][:, j, :],
                                wo_sb[:, j, nsl],
                                start=(j == 0), stop=False)
                        nc.tensor.matmul(y_ps[:, nsl], ones1_sb[:],
                                         boutr_sb[:, nsl],
                                         start=False, stop=True)
                    y_sb = p3.tile([128, DM], F32, tag="y_sb", name=f"ysb{u}")
                    if u % 2 == 0:
                        nc.vector.tensor_copy(y_sb[:], y_ps[:])
                    else:
                        nc.scalar.copy(y_sb[:], y_ps[:])
                    eng = nc.sync if u % 2 == 0 else nc.scalar
                    eng.dma_start(out_d[u * 128:(u + 1) * 128, :], y_sb[:])

    nc.compile()
    return nc


_NC_CACHE = None


def kernel(x, mask, pos_emb, Wq, Wkv, Wout, bout):
    global LAST_EXEC_TIME_NS, LAST_TRACE_DIR, _NC_CACHE

    x = np.asarray(x, dtype=np.float32)
    mask = np.asarray(mask)
    pos_emb = np.asarray(pos_emb, dtype=np.float32)
    Wq = np.asarray(Wq, dtype=np.float32)
    Wkv = np.asarray(Wkv, dtype=np.float32)
    Wout = np.asarray(Wout, dtype=np.float32)
    bout = np.asarray(bout, dtype=np.float32)

    bf = ml_dtypes.bfloat16
    xt = np.ascontiguousarray(x.reshape(R, DM).T).astype(bf)
    wk_full = Wkv[:, :H * DH]
    wv_full = Wkv[:, H * DH:]
    cost = np.ascontiguousarray(np.tile(np.cos(pos_emb).T, (HPC, 1))).astype(bf)
    sint = np.ascontiguousarray(np.tile(np.sin(pos_emb).T, (HPC, 1))).astype(bf)
    maskb = np.ascontiguousarray(
        np.where(mask.reshape(R), 0.0, -1e5).astype(np.float32)
        .reshape(R // 128, 128).T)
    wqr = _rot_cols(Wq)
    wkr = _rot_cols(wk_full)

    in_maps = []
    for c in range(NCORES):
        cols = slice(c * CPC, (c + 1) * CPC)
        in_maps.append({
            "xt": xt,
            "wq": np.ascontiguousarray(Wq[:, cols]).astype(bf),
            "wqr": np.ascontiguousarray(wqr[:, cols]).astype(bf),
            "wk": np.ascontiguousarray(wk_full[:, cols]).astype(bf),
            "wkr": np.ascontiguousarray(wkr[:, cols]).astype(bf),
            "wv": np.ascontiguousarray(wv_full[:, cols]).astype(bf),
            "wout": Wout.astype(bf),
            "boutr": bout[None, :].astype(bf),
            "cost": cost,
            "sint": sint,
            "maskb": maskb,
        })

    dbg = bool(int(os.environ.get("BASS_KERNEL_DEBUG", "0")))
    if _NC_CACHE is None:
        _NC_CACHE = build(dbg=dbg)
    nc = _NC_CACHE

    trace = bool(int(os.environ.get("BASS_KERNEL_TRACE", "0")))
    kwargs = {}
    if trace:
        _install_trace_shim()
        tdir = os.environ.get("BASS_TRACE_DIR", "/tmp/bass_trace_out")
        os.makedirs(tdir, exist_ok=True)
        kwargs["tmpdir"] = tdir
    res = bass_utils.run_bass_kernel_spmd(
        nc, in_maps, core_ids=list(range(NCORES)), trace=trace, **kwargs)
    LAST_EXEC_TIME_NS = res.exec_time_ns
    if res.instructions_and_trace is not None:
        LAST_TRACE_DIR = res.instructions_and_trace[1]
        globals()["LAST_INSTS"] = res.instructions_and_trace[0]

    globals()["LAST_RESULTS"] = res.results
    # core c's out rows: [u*128 + r] = batch u//2, q-row (u%2)*1024 + c*128 + r
    y = np.empty((B, N, DM), dtype=np.float32)
    for c in range(NCORES):
        oc = res.results[c]["out"]
        for u in range(NUNITS):
            b, qh = u // 2, u % 2
            q0 = qh * QHS + c * 128
            y[b, q0:q0 + 128, :] = oc[u * 128:(u + 1) * 128, :]
    return y


# revision 36
# speedup vs baseline: 1.1273x; 1.0156x over previous
"""Distributed multi-head attention kernel for 8 TRN2 NeuronCores.

Module: B=2, N=2048, D_MODEL=1024, H=16, D_HEAD=64 attention with
arbitrary rotary embedding, key-side boolean masking, softmax, and
output projection.

Sharding: head-parallel attention (2 heads per core, both batches),
then one AllToAll per (batch, q-half) to a striped row-parallel layout
for the output projection. Core c owns q-rows [qh*1024 + c*128 ...+128]
of each (batch, q-half) -> 4 x 128 = 512 output rows per core.

Key design points:
 - All matmuls bf16 with fp32 PSUM accumulation.
 - Every matmul lhsT has 128 columns so the compiler's Fast Weight
   Load path stays enabled (65-col weights serialize LDWEIGHTS with
   the matmul, punching holes in PE activity that re-throttle the
   HAM clock gate to K=4/8 for the whole attention phase).
 - attn@v lhsT per (key tile, head) = [v_h(64) | ones(1) | zeros(63)],
   M=128: rows 0..63 = head output, row 64 = softmax denominator.
 - qT/kT produced in [chan, row] layout so scores come out transposed
   [keys, qrows] with keys on partitions.
 - Rotary via host-rotated weight copies: rot2(x@W) == x@Wr.
 - Key mask folded into the softmax exp as a per-partition bias.
 - Attention is software-pipelined by one key tile (scores(kt),
   exp(kt), av(kt-1)) so the PE never waits on the exp of scores it
   just produced; ACT runs back-to-back.
 - One AllToAll per (batch, q-half): [8 shards x 130, 128] bf16 with
   zero wasted shards; the first three overlap compute, only the last
   (~266KB) sits on the tail.
 - Tail: denominator reciprocal broadcast via DRAM round-trip DMA
   (stride-0 partition replication), output bias folded into the
   projection as a ones-row x bias-row accumulation chunk.
"""
import os
import warnings

warnings.filterwarnings("ignore")
import numpy as np
import ml_dtypes

from concourse import bacc, tile, mybir, bass_utils

B, N, DM, H, DH = 2, 2048, 1024, 16, 64
R = B * N
NCORES = 8
HPC = 2
CPC = HPC * DH       # 128 chans per core
KT = 8               # contraction tiles over d_model
RB = 8               # row blocks of 512 over R
NKEYT = 16           # key tiles of 128 over N
QHS = 1024           # qrows per (batch, q-half) unit
NUNITS = B * (N // QHS)  # 4 (b, qh) units
SHARD_ROWS = 2 * (DH + 1)  # 130: [hA 64 | denA 1 | hB 64 | denB 1]

F32 = mybir.dt.float32
BF16 = mybir.dt.bfloat16

LAST_EXEC_TIME_NS = None
LAST_TRACE_DIR = None


def _install_trace_shim():
    import sys
    import types
    import ctypes
    import contextlib

    if "antenv.axon_hooks" in sys.modules:
        return
    so_path = "/opt/axon/libaxon_pjrt.so"
    hook = None
    if os.path.exists(so_path):
        lib = ctypes.CDLL(so_path)
        if hasattr(lib, "axon_start_nrt_profile"):
            lib.axon_start_nrt_profile.argtypes = [
                ctypes.POINTER(ctypes.c_int64), ctypes.c_size_t]
            lib.axon_start_nrt_profile.restype = ctypes.c_int64
            lib.axon_stop_nrt_profile.argtypes = [ctypes.c_char_p]
            lib.axon_stop_nrt_profile.restype = ctypes.c_int64

            @contextlib.contextmanager
            def _hook(output_dir, device_ids):
                import jax
                jax.devices()
                if device_ids:
                    ids = (ctypes.c_int64 * len(device_ids))(*device_ids)
                    rc = lib.axon_start_nrt_profile(ids, len(device_ids))
                else:
                    rc = lib.axon_start_nrt_profile(None, 0)
                if rc != 0:
                    raise RuntimeError(f"axon_start_nrt_profile rc={rc}")
                try:
                    yield
                finally:
                    n = lib.axon_stop_nrt_profile(str(output_dir).encode())
                    print(f"[trace] {n} profile file(s) -> {output_dir}")

            hook = _hook

    mod = types.ModuleType("antenv.axon_hooks")
    mod.get_axon_ntff_profile_hook = lambda: hook
    mod.set_axon_ntff_profile_hook = lambda h: None
    sys.modules["antenv.axon_hooks"] = mod
    bass_utils.upload_artifacts = lambda tmpdir: tmpdir


def _rot_cols(w):
    wr = np.empty_like(w)
    wr[:, 0::2] = -w[:, 1::2]
    wr[:, 1::2] = w[:, 0::2]
    return wr


def build(dbg=False):
    nc = bacc.Bacc("TRN2", target_bir_lowering=False, debug=False,
                   num_devices=NCORES)

    xt_d = nc.dram_tensor("xt", [DM, R], BF16, kind="ExternalInput")
    wq_d = nc.dram_tensor("wq", [DM, CPC], BF16, kind="ExternalInput")
    wqr_d = nc.dram_tensor("wqr", [DM, CPC], BF16, kind="ExternalInput")
    wk_d = nc.dram_tensor("wk", [DM, CPC], BF16, kind="ExternalInput")
    wkr_d = nc.dram_tensor("wkr", [DM, CPC], BF16, kind="ExternalInput")
    wv_d = nc.dram_tensor("wv", [DM, CPC], BF16, kind="ExternalInput")
    wout_d = nc.dram_tensor("wout", [DM, DM], BF16, kind="ExternalInput")
    boutr_d = nc.dram_tensor("boutr", [1, DM], BF16, kind="ExternalInput")
    cost_d = nc.dram_tensor("cost", [CPC, N], BF16, kind="ExternalInput")
    sint_d = nc.dram_tensor("sint", [CPC, N], BF16, kind="ExternalInput")
    maskb_d = nc.dram_tensor("maskb", [128, R // 128], F32, kind="ExternalInput")

    out_d = nc.dram_tensor("out", [NUNITS * 128, DM], F32,
                           kind="ExternalOutput")

    a2a_in = [nc.dram_tensor(f"a2a_in{u}", [NCORES * SHARD_ROWS, 128], BF16)
              for u in range(NUNITS)]
    a2a_out = [nc.dram_tensor(f"a2a_out{u}", [NCORES * SHARD_ROWS, 128], BF16)
               for u in range(NUNITS)]
    recip_d = nc.dram_tensor("recipd", [2 * NCORES, NUNITS * 128], BF16)
    dumm_in = nc.dram_tensor("dummin", [NCORES * 2, 2], BF16)
    dumm_out = nc.dram_tensor("dummout", [NCORES * 2, 2], BF16)

    with tile.TileContext(nc) as tc:
        with tc.tile_pool(name="persist", bufs=1) as pp:
            wq_sb = pp.tile([128, KT, CPC], BF16, tag="wq")
            wqr_sb = pp.tile([128, KT, CPC], BF16, tag="wqr")
            wk_sb = pp.tile([128, KT, CPC], BF16, tag="wk")
            wkr_sb = pp.tile([128, KT, CPC], BF16, tag="wkr")
            wv_sb = pp.tile([128, KT, CPC], BF16, tag="wv")
            cost_sb = pp.tile([CPC, N], BF16, tag="cost")
            sint_sb = pp.tile([CPC, N], BF16, tag="sint")
            maskb_sb = pp.tile([128, R // 128], F32, tag="maskb")
            # per-batch tiles: unit-0 scores must not carry a whole-tile
            # dependency on batch-1's projections
            qt_sb = [pp.tile([CPC, N], BF16, tag=f"qt{b}", name=f"qt{b}")
                     for b in range(B)]
            kt_sb = [pp.tile([CPC, N], BF16, tag=f"kt{b}", name=f"kt{b}")
                     for b in range(B)]
            # [128 keys, key-tile, head, 128]: cols = [v_h | 1 | 0...]
            vaug_sb = [pp.tile([128, NKEYT, HPC, 128], BF16, tag=f"vaug{b}",
                               name=f"vaug{b}") for b in range(B)]
            wo_sb = pp.tile([128, KT, DM], BF16, tag="wo")
            boutr_sb = pp.tile([1, DM], BF16, tag="boutr")
            ones1_sb = pp.tile([1, 128], BF16, tag="ones1")

            def ktview(d):
                return d.ap().rearrange("(k p) n -> p k n", p=128)

            xt_view = xt_d.ap().rearrange("(k p) n -> p k n", p=128)

            # first xt block + weights first so matmuls start early;
            # per-kt pieces across both queues so matmul #0 only waits for
            # its own contraction slice
            xt_sb0 = pp.tile([128, KT, 512], BF16, tag="xt0")
            for kt in range(KT):
                eng = nc.sync if kt % 2 == 0 else nc.scalar
                eng.dma_start(xt_sb0[:, kt, :], xt_view[:, kt, 0:512])
            nc.sync.dma_start(wq_sb[:], ktview(wq_d))
            nc.scalar.dma_start(wqr_sb[:], ktview(wqr_d))
            nc.sync.dma_start(wk_sb[:], ktview(wk_d))
            nc.scalar.dma_start(wkr_sb[:], ktview(wkr_d))
            nc.sync.dma_start(wv_sb[:], ktview(wv_d))
            # zero vaug (cols 65..127 must be 0), set the ones column
            for b in range(B):
                nc.vector.memset(vaug_sb[b][:], 0.0)
                nc.vector.memset(vaug_sb[b][:, :, :, DH:DH + 1], 1.0)
            nc.vector.memset(ones1_sb[:], 1.0)
            # all-zero weights for no-op PE bridge matmuls (accumulate +0)
            zw_sb = pp.tile([128, 128], BF16, tag="zw")
            nc.vector.memset(zw_sb[:], 0.0)
            # dummy collective: absorbs the ~11us first-trigger warmup
            # delay on the CC stream while phase-1 DMAs run
            dz_sb = pp.tile([NCORES * 2, 2], BF16, tag="dz")
            nc.vector.memset(dz_sb[:], 0.0)
            nc.sync.dma_start(dumm_in[:, :], dz_sb[:])
            nc.gpsimd.collective_compute(
                "AllToAll", mybir.AluOpType.bypass,
                replica_groups=[list(range(NCORES))],
                ins=[dumm_in.ap().opt()], outs=[dumm_out.ap().opt()])
            # pre-load the ACT Exp table during the initial DMA wait so the
            # first real softmax exp doesn't stall the pipeline
            warm_sb = pp.tile([1, 2], F32, tag="warm")
            nc.vector.memset(warm_sb[:], 0.0)
            nc.scalar.activation(warm_sb[0:1, 1:2], warm_sb[0:1, 0:1],
                                 mybir.ActivationFunctionType.Exp)
            nc.scalar.dma_start(cost_sb[:], cost_d[:, :])
            nc.scalar.dma_start(sint_sb[:], sint_d[:, :])
            nc.scalar.dma_start(maskb_sb[:], maskb_d[:, :])

            # ---- Phase 1: projections + rotary + v_aug ----
            with tc.tile_pool(name="p1", bufs=2) as p1, \
                 tc.tile_pool(name="ps1", bufs=1, space="PSUM") as ps1:
                for rb in range(RB):
                    c0 = rb * 512
                    if rb == 0:
                        xt_sb = xt_sb0
                    else:
                        xt_sb = p1.tile([128, KT, 512], BF16, tag="xt")
                        if rb == 4:
                            eng = nc.gpsimd
                        elif rb % 2 == 1:
                            eng = nc.sync
                        else:
                            eng = nc.scalar
                        eng.dma_start(xt_sb[:], xt_view[:, :, c0:c0 + 512])

                    q_ps = ps1.tile([128, 512], F32, tag="q")
                    qr_ps = ps1.tile([128, 512], F32, tag="qr")
                    k_ps = ps1.tile([128, 512], F32, tag="k")
                    kr_ps = ps1.tile([128, 512], F32, tag="kr")
                    v_ps = ps1.tile([128, 512], F32, tag="v")
                    for kt in range(KT):
                        st, sp = kt == 0, kt == KT - 1
                        for ps_t, w_t in [(q_ps, wq_sb), (qr_ps, wqr_sb),
                                          (k_ps, wk_sb), (kr_ps, wkr_sb)]:
                            nc.tensor.matmul(ps_t[:], w_t[:, kt, :],
                                             xt_sb[:, kt, :], start=st, stop=sp)
                        for vt in range(4):
                            nc.tensor.matmul(
                                v_ps[:, vt * 128:(vt + 1) * 128],
                                xt_sb[:, kt, vt * 128:(vt + 1) * 128],
                                wv_sb[:, kt, :], start=(st and vt == 0), stop=sp)

                    b = rb // 4
                    cc = c0 % N
                    tmp = p1.tile([128, 512], BF16, tag="rottmp")
                    for dst, a_ps, b_ps in [(qt_sb[b], q_ps, qr_ps),
                                            (kt_sb[b], k_ps, kr_ps)]:
                        dv = dst[:, cc:cc + 512]
                        nc.vector.tensor_mul(dv, a_ps[:], cost_sb[:, cc:cc + 512])
                        nc.vector.tensor_mul(tmp[:], b_ps[:], sint_sb[:, cc:cc + 512])
                        nc.vector.tensor_add(dv, dv, tmp[:])

                    kt0 = (rb % 4) * 4
                    vp = v_ps[:].rearrange("p (t c) -> p t c", c=128)
                    nc.vector.tensor_copy(
                        vaug_sb[b][:, kt0:kt0 + 4, 0, 0:DH], vp[:, :, 0:DH])
                    nc.vector.tensor_copy(
                        vaug_sb[b][:, kt0:kt0 + 4, 1, 0:DH],
                        vp[:, :, DH:2 * DH])

                # keep PE busy across the phase transition (an idle gap
                # here re-throttles the PE clock for the rest of the run)
                brid_ps = ps1.tile([128, 512], F32, tag="brid")
                for i in range(12):
                    nc.tensor.matmul(brid_ps[:], wq_sb[:, i % KT, :],
                                     xt_sb0[:, i % KT, :],
                                     start=(i == 0), stop=(i == 11))

            # wout needed only in phase 3 — load it behind phase-1 traffic
            nc.scalar.dma_start(wo_sb[:], wout_d.ap().rearrange(
                "(k p) n -> p k n", p=128))
            nc.sync.dma_start(boutr_sb[:], boutr_d[:, :])

            # ---- per-unit tail prep: gather + normalize, SBUF/DMA only.
            # prep(u) is emitted one unit late so its DMAs never
            # head-of-line-block attention; nothing here touches the ACT
            # queue (phase-2 exps must stream uninterrupted) or PSUM.
            onorm_tiles = [None] * NUNITS

            def tail_prep(u):
                # units 2/3 are emitted after the last exp, so the ACT
                # queue is free to carry their DMAs too; gpsimd only for
                # unit 3 (its queue is blocked on collective completions)
                if u == NUNITS - 1:
                    engs = [nc.sync, nc.scalar, nc.gpsimd]
                elif u == NUNITS - 2:
                    engs = [nc.sync, nc.scalar]
                else:
                    engs = [nc.sync]
                ne = len(engs)
                av = a2a_out[u].ap().rearrange("(j r) n -> r j n",
                                               r=SHARD_ROWS)
                o_t = pp.tile([128, NCORES, 128], BF16, tag=f"ot{u}",
                              name=f"ot{u}")
                den_t = pp.tile([2 * NCORES, 128], BF16, tag=f"dt{u}",
                                name=f"dt{u}")
                engs[0].dma_start(den_t[0:NCORES, :], av[DH:DH + 1, :, :])
                engs[1 % ne].dma_start(den_t[NCORES:2 * NCORES, :],
                                       av[CPC + 1:CPC + 2, :, :])
                engs[0].dma_start(o_t[0:DH, :, :], av[0:DH, :, :])
                engs[2 % ne].dma_start(o_t[DH:CPC, :, :],
                                       av[DH + 1:CPC + 1, :, :])
                recipf = pp.tile([2 * NCORES, 128], F32, tag=f"rf{u}",
                                 name=f"rf{u}")
                recipb16 = pp.tile([2 * NCORES, 128], BF16, tag=f"rb{u}",
                                   name=f"rb{u}")
                nc.vector.reciprocal(recipf[:], den_t[:])
                nc.vector.tensor_copy(recipb16[:], recipf[:])
                engs[1 % ne].dma_start(recip_d[:, u * 128:(u + 1) * 128],
                                       recipb16[:])
                recipb = pp.tile([128, NCORES, 128], BF16, tag=f"rc{u}",
                                 name=f"rc{u}")
                rv = recip_d.ap()
                # one broadcast DMA per head-half: [8 j-rows, 128] fans out
                # across 64 partitions (stride-0), 2 descriptors per unit
                # instead of 16 small DMAs contending with the A2As
                for h in range(HPC):
                    engs[h % ne].dma_start(
                        recipb[h * DH:(h + 1) * DH, :, :],
                        rv[NCORES * h:NCORES * h + NCORES,
                           u * 128:(u + 1) * 128][None, :, :]
                        .to_broadcast((DH, NCORES, 128)))
                onorm = pp.tile([128, NCORES, 128], BF16, tag=f"on{u}",
                                name=f"on{u}")
                nc.vector.tensor_mul(onorm[:], o_t[:], recipb[:])
                onorm_tiles[u] = onorm

            # ---- Phase 2: attention, two heads packed, per (b, q-half) ----
            with tc.tile_pool(name="p2", bufs=2) as p2, \
                 tc.tile_pool(name="ps_sc", bufs=1, space="PSUM") as ps_sc, \
                 tc.tile_pool(name="ps_o", bufs=1, space="PSUM") as ps_o:
                for u in range(NUNITS):
                    b, qh = u // 2, u % 2
                    qbase = qh * QHS
                    o_ps = [ps_o.tile([128, QHS], F32, tag=f"outp{h}",
                                      name=f"ops{h}") for h in range(HPC)]
                    # software-pipelined by one key tile: emit scores(kt),
                    # exp(kt), av(kt-1) so the PE never waits on the exp of
                    # scores it just produced; ACT stays back-to-back.
                    prev_p = None
                    for kt in range(NKEYT + 1):
                        if kt < NKEYT:
                            g = b * NKEYT + kt
                            krow = kt * 128
                            sc = [ps_sc.tile([128, QHS], F32, tag=f"sc{h}",
                                             name=f"sc{h}") for h in range(HPC)]
                            # interleave heads so the PE runs them in
                            # different row groups concurrently
                            for qq in range(QHS // 512):
                                for h in range(HPC):
                                    ho = h * DH
                                    nc.tensor.matmul(
                                        sc[h][:, qq * 512:(qq + 1) * 512],
                                        kt_sb[b][ho:ho + DH, krow:krow + 128],
                                        qt_sb[b][ho:ho + DH,
                                                 qbase + qq * 512:qbase + (qq + 1) * 512],
                                        start=True, stop=True)
                            p_sb = []
                            for h in range(HPC):
                                pt = p2.tile([128, QHS], BF16, tag=f"p{h}",
                                             name=f"pt{h}")
                                nc.scalar.activation(
                                    pt[:], sc[h][:],
                                    mybir.ActivationFunctionType.Exp,
                                    bias=maskb_sb[:, g:g + 1],
                                    scale=float(DH ** -0.5))
                                p_sb.append(pt)
                        if kt == NKEYT or (u == 0 and 1 <= kt <= 3):
                            # no-op bridge (+0 accumulate) spans the final
                            # exp wait (and unit-0's ramp-in) so the HAM
                            # clock gate stays warm
                            for i in range(4 if kt == NKEYT else 2):
                                nc.tensor.matmul(
                                    o_ps[0][:, 0:512], zw_sb[:],
                                    qt_sb[0][:, 0:512],
                                    start=False, stop=False)
                        if prev_p is not None:
                            pkt = kt - 1
                            for h in range(HPC):
                                va_l = vaug_sb[b][:, pkt, h, :]
                                for qq in range(QHS // 512):
                                    nc.tensor.matmul(
                                        o_ps[h][:, qq * 512:(qq + 1) * 512],
                                        va_l,
                                        prev_p[h][:, qq * 512:(qq + 1) * 512],
                                        start=(pkt == 0),
                                        stop=(pkt == NKEYT - 1))
                        if kt < NKEYT:
                            prev_p = p_sb

                    # stage [hA|denA|hB|denB] shards and fire this unit's A2A
                    onb = [p2.tile([DH + 1, QHS], BF16, tag=f"onb{h}",
                                   name=f"onb{h}") for h in range(HPC)]
                    for h in range(HPC):
                        nc.vector.tensor_copy(onb[h][:], o_ps[h][0:DH + 1, :])
                    # staging off the scalar queue while exps still stream
                    # (a scalar-queue DMA would head-of-line-block them);
                    # the last unit uses the now-idle ACT queue for speed
                    seng = nc.scalar if u == NUNITS - 1 else nc.sync
                    for d in range(NCORES):
                        r0 = d * SHARD_ROWS
                        cs = slice(d * 128, (d + 1) * 128)
                        seng.dma_start(a2a_in[u][r0:r0 + DH + 1, :],
                                       onb[0][:, cs])
                        seng.dma_start(
                            a2a_in[u][r0 + DH + 1:r0 + SHARD_ROWS, :],
                            onb[1][:, cs])
                    nc.gpsimd.collective_compute(
                        "AllToAll", mybir.AluOpType.bypass,
                        replica_groups=[list(range(NCORES))],
                        ins=[a2a_in[u].ap().opt()],
                        outs=[a2a_out[u].ap().opt()])
                    if u >= 1:
                        tail_prep(u - 1)

            # ---- Phase 3: output projections; units 0-2 are ready the
            # moment the PSUM pools free, so the PE rolls straight from
            # attention into a dense warm matmul chain; only unit 3's
            # A2A+prep chain is exposed.
            with tc.tile_pool(name="p3", bufs=2) as p3, \
                 tc.tile_pool(name="ps3", bufs=2, space="PSUM") as ps3:
                for u in range(NUNITS):
                    if u == NUNITS - 1:
                        # unit 3's prep emitted only now so nothing it
                        # waits on (the last A2A) can delay units 0-2
                        tail_prep(u)
                    y_ps = ps3.tile([128, DM], F32, tag="y")
                    for nb in range(2):
                        nsl = slice(nb * 512, (nb + 1) * 512)
                        for j in range(NCORES):
                            nc.tensor.matmul(
                                y_ps[:, nsl], onorm_tiles[u][:, j, :],
                                wo_sb[:, j, nsl],
                                start=(j == 0), stop=False)
                        nc.tensor.matmul(y_ps[:, nsl], ones1_sb[:],
                                         boutr_sb[:, nsl],
                                         start=False, stop=True)
                    y_sb = p3.tile([128, DM], F32, tag="y_sb", name=f"ysb{u}")
                    if u % 2 == 0:
                        nc.vector.tensor_copy(y_sb[:], y_ps[:])
                    else:
                        nc.scalar.copy(y_sb[:], y_ps[:])
                    eng = nc.sync if u % 2 == 0 else nc.scalar
                    eng.dma_start(out_d[u * 128:(u + 1) * 128, :], y_sb[:])

    nc.compile()
    return nc


_NC_CACHE = None


def kernel(x, mask, pos_emb, Wq, Wkv, Wout, bout):
    global LAST_EXEC_TIME_NS, LAST_TRACE_DIR, _NC_CACHE

    x = np.asarray(x, dtype=np.float32)
    mask = np.asarray(mask)
    pos_emb = np.asarray(pos_emb, dtype=np.float32)
    Wq = np.asarray(Wq, dtype=np.float32)
    Wkv = np.asarray(Wkv, dtype=np.float32)
    Wout = np.asarray(Wout, dtype=np.float32)
    bout = np.asarray(bout, dtype=np.float32)

    bf = ml_dtypes.bfloat16
    xt = np.ascontiguousarray(x.reshape(R, DM).T).astype(bf)
    wk_full = Wkv[:, :H * DH]
    wv_full = Wkv[:, H * DH:]
    cost = np.ascontiguousarray(np.tile(np.cos(pos_emb).T, (HPC, 1))).astype(bf)
    sint = np.ascontiguousarray(np.tile(np.sin(pos_emb).T, (HPC, 1))).astype(bf)
    maskb = np.ascontiguousarray(
        np.where(mask.reshape(R), 0.0, -1e5).astype(np.float32)
        .reshape(R // 128, 128).T)
    wqr = _rot_cols(Wq)
    wkr = _rot_cols(wk_full)

    in_maps = []
    for c in range(NCORES):
        cols = slice(c * CPC, (c + 1) * CPC)
        in_maps.append({
            "xt": xt,
            "wq": np.ascontiguousarray(Wq[:, cols]).astype(bf),
            "wqr": np.ascontiguousarray(wqr[:, cols]).astype(bf),
            "wk": np.ascontiguousarray(wk_full[:, cols]).astype(bf),
            "wkr": np.ascontiguousarray(wkr[:, cols]).astype(bf),
            "wv": np.ascontiguousarray(wv_full[:, cols]).astype(bf),
            "wout": Wout.astype(bf),
            "boutr": bout[None, :].astype(bf),
            "cost": cost,
            "sint": sint,
            "maskb": maskb,
        })

    dbg = bool(int(os.environ.get("BASS_KERNEL_DEBUG", "0")))
    if _NC_CACHE is None:
        _NC_CACHE = build(dbg=dbg)
    nc = _NC_CACHE

    trace = bool(int(os.environ.get("BASS_KERNEL_TRACE", "0")))
    kwargs = {}
    if trace:
        _install_trace_shim()
        tdir = os.environ.get("BASS_TRACE_DIR", "/tmp/bass_trace_out")
        os.makedirs(tdir, exist_ok=True)
        kwargs["tmpdir"] = tdir
    res = bass_utils.run_bass_kernel_spmd(
        nc, in_maps, core_ids=list(range(NCORES)), trace=trace, **kwargs)
    LAST_EXEC_TIME_NS = res.exec_time_ns
    if res.instructions_and_trace is not None:
        LAST_TRACE_DIR = res.instructions_and_trace[1]
        globals()["LAST_INSTS"] = res.instructions_and_trace[0]

    globals()["LAST_RESULTS"] = res.results
    # core c's out rows: [u*128 + r] = batch u//2, q-row (u%2)*1024 + c*128 + r
    y = np.empty((B, N, DM), dtype=np.float32)
    for c in range(NCORES):
        oc = res.results[c]["out"]
        for u in range(NUNITS):
            b, qh = u // 2, u % 2
            q0 = qh * QHS + c * 128
            y[b, q0:q0 + 128, :] = oc[u * 128:(u + 1) * 128, :]
    return y
